# revision 35
# baseline (speedup 1.0000x reference)
"""Multi-head causal self-attention (B=2, T=2048, D=2048, H=16) on 8 Trainium2
NeuronCores.

Sharding: core c handles batch b = c//4 and 4 heads hs = 4*(c%4) .. hs+4
(batch x tensor-parallel heads). Each core computes Q/K/V projections for its
head slice, causal attention for its 4 heads, and a row-parallel partial of the
output projection (out_heads_slice @ wo_slice.T). The 4 partials per batch are
summed on the host (wo row-parallel reduce); bo is added on one core per batch.

Device schedule (single pass over x; all engine streams interleaved):
 - x is fed pre-transposed per batch: xT [D, T]. For each 512-token chunk t4,
   A(t4) computes QT/KT ([hd, t], transposed) and V' ([t, hd] + ones column)
   for that chunk, streaming xT chunk-by-chunk exactly once.
 - att(t4) runs causal attention for chunk t4's queries over k-blocks
   0..4*t4+3: scores are computed transposed as ST[k, q], exp runs on the
   Scalar engine WITHOUT max subtraction (scores are O(10), f32 exp is safe),
   row sums come free via a ones-column appended to V'. Diagonal 128x128
   blocks are masked by an upper-triangular 0/1 multiply; blocks above the
   diagonal are never computed.
 - O = P~ @ V' lands as [q, hd | rowsum]; normalize by the reciprocal rowsum,
   PE-transpose to OT [hd, q] for the output projection.
 - Emission interleaves A(t4+1) with att(t4) cost-proportionally so the PE
   always has projection matmuls to run while the Scalar engine works through
   the exps; fin (output-projection) blocks for chunk t4-1 are merged into
   att(t4) as extra filler, and t4=3's own fins interleave into the last
   head's PV chains so the PE never drains until the very end.
 - Output partials are written bf16 (halves the output DMA); the host sums
   the 4 partials per batch in f64.

All matmul inputs are bf16 (PSUM accumulation in f32).
"""

import sys
import numpy as np

if '/opt/trn_rl_repo' not in sys.path:
    sys.path.insert(0, '/opt/trn_rl_repo')

import ml_dtypes
from contextlib import ExitStack

import concourse.mybir as mybir
import concourse.tile as tile
from concourse import bacc
from concourse.bass_utils import run_bass_kernel_spmd

B, T, D, H = 2, 2048, 2048, 16
HD = 128           # head dim
P = 128            # partitions
HPC = 4            # heads per core
NCORES = 8
SCALE = float(HD) ** -0.5
DC = D // P        # 16 contraction chunks for projections
NT = T // P        # 16 t-chunks of 128
QT_TILES = T // 512  # 4 q tiles of 512

BF16 = mybir.dt.bfloat16
F32 = mybir.dt.float32
NPBF16 = ml_dtypes.bfloat16

_BUILD_CACHE = {}


def _merge(ua, ub):
    """Cost-proportional merge of two (cost, fn) unit lists."""
    ta = sum(c for c, _ in ua) or 1
    tb = sum(c for c, _ in ub) or 1
    out = []
    ia = ib = 0
    ca = cb = 0
    while ia < len(ua) or ib < len(ub):
        if ib >= len(ub) or (ia < len(ua) and ca * tb <= cb * ta):
            c, f = ua[ia]; ia += 1; ca += c
        else:
            c, f = ub[ib]; ib += 1; cb += c
        out.append((c, f))
    return out


def _build():
    """Build the per-core Bass program (identical across cores; data differs)."""
    nc = bacc.Bacc("TRN2", target_bir_lowering=False, debug=False)

    xT = nc.dram_tensor("xT", [D, T], BF16, kind="ExternalInput").ap()
    wqT = nc.dram_tensor("wqT", [D, HPC * HD], BF16, kind="ExternalInput").ap()
    wkT = nc.dram_tensor("wkT", [D, HPC * HD], BF16, kind="ExternalInput").ap()
    wvT = nc.dram_tensor("wvT", [D, HPC * HD], BF16, kind="ExternalInput").ap()
    woT = nc.dram_tensor("woT", [HPC * HD, D], BF16, kind="ExternalInput").ap()
    bq = nc.dram_tensor("bq", [P, HPC], F32, kind="ExternalInput").ap()
    bk = nc.dram_tensor("bk", [P, HPC], F32, kind="ExternalInput").ap()
    bv = nc.dram_tensor("bv", [P, HPC * HD], F32, kind="ExternalInput").ap()
    bo = nc.dram_tensor("bo", [P, D], F32, kind="ExternalInput").ap()
    tri = nc.dram_tensor("tri", [P, P], BF16, kind="ExternalInput").ap()
    ident = nc.dram_tensor("ident", [P, P], BF16, kind="ExternalInput").ap()
    out = nc.dram_tensor("out", [T, D], BF16, kind="ExternalOutput").ap()

    with tile.TileContext(nc) as tc:
        with ExitStack() as ctx:
            persist = ctx.enter_context(tc.tile_pool(name="persist", bufs=1))

            wq_sb = persist.tile([P, DC, HPC * HD], BF16, name="wq_sb")
            wk_sb = persist.tile([P, DC, HPC * HD], BF16, name="wk_sb")
            wv_sb = persist.tile([P, DC, HPC * HD], BF16, name="wv_sb")
            wo_sb = persist.tile([P, HPC, D], BF16, name="wo_sb")
            bq_sb = persist.tile([P, HPC], F32, name="bq_sb")
            bk_sb = persist.tile([P, HPC], F32, name="bk_sb")
            bv3_sb = persist.tile([P, HPC, HD], F32, name="bv3_sb")
            bo_sb = persist.tile([P, D], F32, name="bo_sb")
            tri_sb = persist.tile([P, P], BF16, name="tri_sb")
            id_sb = persist.tile([P, P], BF16, name="id_sb")
            QT_sb = persist.tile([P, HPC, T], BF16, name="QT_sb")
            KT_sb = persist.tile([P, HPC, T], BF16, name="KT_sb")
            # V' with ones column: [t-within-chunk, head, t-chunk, hd+1]
            VP_sb = persist.tile([P, HPC, NT, HD + 1], BF16, name="VP_sb")
            OT_sb = persist.tile([P, HPC, NT, P], BF16, name="OT_sb")

            # warmup scratch must be first on the gpsimd queue — anything later
            # sits behind the whole weight-DMA stream and stalls the PE
            scr = persist.tile([P, P], BF16, name="scr")
            nc.gpsimd.memset(scr[:], 1.0)

            # weight/bias DMAs in the order A(0) consumes them; the d=0 chunk
            # of wq is split per head so the first matmul's weights land ASAP
            for h in range(HPC):
                nc.gpsimd.dma_start(wq_sb[:, 0, h * HD:(h + 1) * HD],
                                    wqT[0:P, h * HD:(h + 1) * HD])
            for d in range(1, DC):
                nc.gpsimd.dma_start(wq_sb[:, d, :], wqT[d * P:(d + 1) * P, :])
            nc.gpsimd.dma_start(bq_sb[:], bq[:])
            nc.gpsimd.dma_start(bk_sb[:], bk[:])
            for d in range(DC):
                nc.gpsimd.dma_start(wk_sb[:, d, :], wkT[d * P:(d + 1) * P, :])
            nc.gpsimd.dma_start(tri_sb[:], tri[:])
            nc.gpsimd.dma_start(id_sb[:], ident[:])
            for hh in range(HPC):
                nc.gpsimd.dma_start(wo_sb[:, hh, :], woT[hh * P:(hh + 1) * P, :])
            nc.gpsimd.dma_start(bo_sb[:], bo[:])
            nc.gpsimd.memset(VP_sb[:, :, :, HD:HD + 1], 1.0)

            def load_vw():
                # wv/bv ride the sync queue right after A(0)'s x chunks, so the
                # V phase never waits behind wq/wk on the gpsimd queue
                for h in range(HPC):
                    nc.sync.dma_start(bv3_sb[:, h, :], bv[:, h * HD:(h + 1) * HD])
                for d in range(DC):
                    nc.sync.dma_start(wv_sb[:, d, :], wvT[d * P:(d + 1) * P, :])

            work = persist
            ax = work

            xas = {}  # t4 -> list of 16 x chunks

            def load_x(t4):
                xs = []
                for d in range(DC):
                    xa = ax.tile([P, 512], BF16, tag="xa", bufs=28, name=f"xa{t4}_{d}")
                    if t4 == 0 and d == 0:
                        nc.sync.dma_start(xa[:, 0:256], xT[0:P, 0:256])
                        nc.sync.dma_start(xa[:, 256:512], xT[0:P, 256:512])
                    else:
                        nc.sync.dma_start(xa[:], xT[d * P:(d + 1) * P, t4 * 512:(t4 + 1) * 512])
                    xs.append(xa)
                xas[t4] = xs

            def q_bias(t4, h, ps):
                nc.vector.tensor_scalar_add(
                    QT_sb[:, h, t4 * 512:(t4 + 1) * 512], ps[:], bq_sb[:, h:h + 1])

            def k_bias(t4, h, ps):
                nc.vector.tensor_scalar_add(
                    KT_sb[:, h, t4 * 512:(t4 + 1) * 512], ps[:], bk_sb[:, h:h + 1])

            def v_bias(t4, j, ps):
                # ps is [P, HPC, HD]; one strided DVE add covers all 4 heads
                kb = 4 * t4 + j
                nc.vector.tensor_add(out=VP_sb[:, :, kb, 0:HD],
                                     in0=ps[:], in1=bv3_sb[:])

            # warm the Scalar engine's Exp table during A(0) so the first real
            # activation doesn't eat the 1.3us ACT_TABLE_LOAD
            wrm = work.tile([P, 1], F32, tag="rec", bufs=4, name="wrm")
            nc.scalar.activation(wrm[:], scr[:, 0:1],
                                 mybir.ActivationFunctionType.Exp, scale=1.0)

            aps = ctx.enter_context(tc.tile_pool(name="ps", bufs=2, space="PSUM"))
            cps = aps

            # ---- A(0): d-outer, borrowing the idle st/fin tag banks so no
            #      dedicated pool (and no mid-program pool-close barrier) is
            #      needed; subsequent chunks use the 2-slot qkv tag only ----
            def a0_tile(tag, name):
                return aps.tile([P, 512], F32, tag=tag, bufs=2, name=name)

            # PE p-state warmup on memset data while the first weight/x
            # DMAs are still landing
            dmy = cps.tile([P, HD + 1], F32, tag="o", bufs=2, name="dmy")
            for _ in range(24):
                nc.tensor.matmul(dmy[:, 0:P], scr[:], scr[:], start=True, stop=True)
            load_x(0)
            load_vw()
            xs = xas[0]
            psq = [a0_tile(t, f"a0q{h}") for h, t in enumerate(['qkv', 'qkv', 'st', 'st'])]
            for d in range(DC):
                st, sp = (d == 0), (d == DC - 1)
                for h in range(HPC):
                    nc.tensor.matmul(psq[h][:], wq_sb[:, d, h * HD:(h + 1) * HD],
                                     xs[d][:], start=st, stop=sp)
                if 1 <= d <= 6:
                    # keep the DVFS ramp alive while the weight stream trickles in
                    for _ in range(2):
                        nc.tensor.matmul(dmy[:, 0:P], scr[:], scr[:], start=True, stop=True)
            nc.vector.tensor_copy(out=scr[:], in_=dmy[:, 0:P])
            for h in range(HPC):
                q_bias(0, h, psq[h])
            psk = [a0_tile(t, f"a0k{h}") for h, t in enumerate(['fin', 'fin', 'qkv', 'qkv'])]
            for d in range(DC):
                st, sp = (d == 0), (d == DC - 1)
                for h in range(HPC):
                    nc.tensor.matmul(psk[h][:], wk_sb[:, d, h * HD:(h + 1) * HD],
                                     xs[d][:], start=st, stop=sp)
            for h in range(HPC):
                k_bias(0, h, psk[h])
            # V is d-outer as well: consume each wv chunk as it lands
            # instead of demanding all 16 for the first chain
            psv = [aps.tile([P, HPC, HD], F32, tag=t, bufs=2, name=f"a0v{j}")
                   for j, t in enumerate(['st', 'st', 'fin', 'fin'])]
            for d in range(DC):
                st, sp = (d == 0), (d == DC - 1)
                for j in range(4):
                    nc.tensor.matmul(psv[j][:], xs[d][:, j * P:(j + 1) * P],
                                     wv_sb[:, d, :], start=st, stop=sp)
            for j in range(4):
                v_bias(0, j, psv[j])

            # ---- unit builders ----
            def a_units(t4):
                """Projection chunk t4 as (cost, fn) units; chains are
                emitted contiguously so only one qkv PSUM bank is open."""
                units = [(0, lambda: load_x(t4))]

                def chain(kind, idx):
                    shape = [P, HPC, HD] if kind == 'v' else [P, 512]
                    ps = aps.tile(shape, F32, tag="qkv", bufs=2,
                                  name=f"ps{kind}{t4}_{idx}")
                    def sub(d4):
                        def f():
                            for dd in range(4):
                                d = 4 * d4 + dd
                                st, sp = (d == 0), (d == DC - 1)
                                if kind == 'v':
                                    nc.tensor.matmul(ps[:], xas[t4][d][:, idx * P:(idx + 1) * P],
                                                     wv_sb[:, d, :], start=st, stop=sp)
                                else:
                                    w = wq_sb if kind == 'q' else wk_sb
                                    nc.tensor.matmul(ps[:], w[:, d, idx * HD:(idx + 1) * HD],
                                                     xas[t4][d][:], start=st, stop=sp)
                        return f
                    for d4 in range(4):
                        units.append((880, sub(d4)))
                    bias = {'q': q_bias, 'k': k_bias, 'v': v_bias}[kind]
                    units.append((0, lambda: bias(t4, idx, ps)))

                for kind in 'qkv':
                    for idx in range(4):
                        chain(kind, idx)
                return units

            def fin_units(ft4):
                """Output-projection blocks for chunk ft4 (one per (qs, n));
                the 4 n-chunks land in one [P, 2048] tile written out by a
                single wide DMA (4KB descriptor lines, 16 DMAs total)."""
                units = []
                obs = {}
                for qs in range(4):
                    tch = 4 * ft4 + qs
                    for n in range(4):
                        def f(tch=tch, n=n):
                            fin = cps.tile([P, 512], F32, tag="fin", bufs=2,
                                           name=f"fin{tch}_{n}")
                            for hh in range(HPC):
                                nc.tensor.matmul(fin[:], OT_sb[:, hh, tch, :],
                                                 wo_sb[:, hh, n * 512:(n + 1) * 512],
                                                 start=(hh == 0), stop=(hh == HPC - 1))
                            if n == 0:
                                obs[tch] = work.tile([P, D], BF16, tag="ob", bufs=3,
                                                     name=f"ob{tch}")
                            nc.vector.tensor_add(out=obs[tch][:, n * 512:(n + 1) * 512],
                                                 in0=fin[:],
                                                 in1=bo_sb[:, n * 512:(n + 1) * 512])
                            eng = nc.gpsimd if tch % 2 == 0 else nc.sync
                            if ft4 == 3 and n == 1:
                                # final groups: write halves as they complete so
                                # the last transfer is small
                                eng.dma_start(out[tch * P:(tch + 1) * P, 0:1024],
                                              obs[tch][:, 0:1024])
                            elif ft4 == 3 and n == 3:
                                if tch == NT - 1:
                                    # very last write: two row-halves on both
                                    # queues so the final transfer is ~0.6us
                                    nc.gpsimd.dma_start(
                                        out[tch * P:tch * P + 64, 1024:D],
                                        obs[tch][0:64, 1024:D])
                                    nc.sync.dma_start(
                                        out[tch * P + 64:(tch + 1) * P, 1024:D],
                                        obs[tch][64:P, 1024:D])
                                else:
                                    eng.dma_start(out[tch * P:(tch + 1) * P, 1024:D],
                                                  obs[tch][:, 1024:D])
                            elif ft4 < 3 and n == 3:
                                eng.dma_start(out[tch * P:(tch + 1) * P, :], obs[tch][:])
                        units.append((950, f))
                return units

            def att_units(t4):
                """Attention for q-tile t4. For t4=3 the per-qs fins are
                interleaved into the last head's PV chains."""
                units = []
                kmax = 4 * t4 + 4
                last_fins = fin_units(3) if t4 == 3 else None

                def head_units(h):
                    hu = []
                    pts = []
                    qoffs = []
                    for kb in range(kmax):
                        qoff = max(0, kb - 4 * t4) * P
                        w = 512 - qoff
                        def f(kb=kb, qoff=qoff, w=w):
                            stp = cps.tile([P, 512], F32, tag="st", bufs=2,
                                           name=f"st{t4}_{h}_{kb}")
                            pt = work.tile([P, 512], BF16, tag="pt", bufs=20,
                                           name=f"pt{t4}_{h}_{kb}")
                            nc.tensor.matmul(stp[:, 0:w], KT_sb[:, h, kb * P:(kb + 1) * P],
                                             QT_sb[:, h, t4 * 512 + qoff:(t4 + 1) * 512],
                                             start=True, stop=True)
                            nc.scalar.activation(pt[:, 0:w], stp[:, 0:w],
                                                 mybir.ActivationFunctionType.Exp,
                                                 scale=SCALE)
                            if kb >= 4 * t4:
                                nc.vector.tensor_mul(out=pt[:, 0:P], in0=pt[:, 0:P],
                                                     in1=tri_sb[:])
                            pts.append(pt)
                            qoffs.append(qoff)
                        hu.append((int(w * 0.43) + 160, f))

                    osbs = {}

                    def pv_chain(qs):
                        qb = 4 * t4 + qs
                        klim = qb + 1
                        ops = cps.tile([P, HD + 1], F32, tag="o", bufs=2,
                                       name=f"o{t4}_{h}_{qs}")
                        for kb in range(klim):
                            c0 = qs * P - qoffs[kb]
                            nc.tensor.matmul(ops[:], pts[kb][:, c0:c0 + P],
                                             VP_sb[:, h, kb, :],
                                             start=(kb == 0), stop=(kb == klim - 1))
                        rec = work.tile([P, 1], F32, tag="rec", bufs=4,
                                        name=f"rec{t4}_{h}_{qs}")
                        nc.vector.reciprocal(rec[:], ops[:, HD:HD + 1])
                        osb = work.tile([P, HD], BF16, tag="osb", bufs=4,
                                        name=f"osb{t4}_{h}_{qs}")
                        nc.vector.tensor_scalar_mul(osb[:], ops[:, 0:HD], rec[:])
                        osbs[qs] = osb

                    def o_transpose(qs):
                        qb = 4 * t4 + qs
                        tp2 = cps.tile([P, P], BF16, tag="o", bufs=2,
                                       name=f"tpo{t4}_{h}_{qs}")
                        nc.tensor.transpose(tp2[:], osbs[qs][:], id_sb[:])
                        nc.vector.tensor_copy(out=OT_sb[:, h, qb, :], in_=tp2[:])

                    # deferred transpose: tr(qs-1) after pv(qs) so the PE
                    # never waits on the DVE normalize of the current chunk.
                    # For the very last head, tr(3) moves BEFORE the qs=2 fins
                    # so its OT copy isn't queued on the DVE behind their ob
                    # adds (which would stall the final fin block ~3us).
                    fin_tail = last_fins is not None and h == HPC - 1
                    for qs in range(4):
                        hu.append(((4 * t4 + qs + 1) * 115 + 80,
                                   lambda qs=qs: pv_chain(qs)))
                        if qs >= 1:
                            hu.append((160, lambda qs=qs: o_transpose(qs - 1)))
                            if fin_tail and qs < 3:
                                hu.extend(last_fins[4 * (qs - 1):4 * qs])
                    hu.append((160, lambda: o_transpose(3)))
                    if fin_tail:
                        hu.extend(last_fins[8:16])
                    return hu

                for h in range(HPC):
                    units.extend(head_units(h))
                return units

            # ---- top-level schedule ----
            stream = _merge(att_units(0), a_units(1))
            stream += _merge(_merge(att_units(1), fin_units(0)), a_units(2))
            stream += _merge(_merge(att_units(2), fin_units(1)), a_units(3))
            stream += _merge(att_units(3), fin_units(2))
            for _, f in stream:
                f()

    nc.compile()
    return nc


def _get_program():
    if 'p' not in _BUILD_CACHE:
        _BUILD_CACHE['p'] = _build()
    return _BUILD_CACHE['p']


def _prep_in_maps(x, wq, bq, wk, bk, wv, bv, wo, bo):
    xbf = np.asarray(x, dtype=np.float32).astype(NPBF16)
    tri = np.triu(np.ones((P, P), dtype=np.float32)).astype(NPBF16)
    ident = np.eye(P, dtype=np.float32).astype(NPBF16)
    wqbf = np.asarray(wq, dtype=np.float32).astype(NPBF16)
    wkbf = np.asarray(wk, dtype=np.float32).astype(NPBF16)
    wvbf = np.asarray(wv, dtype=np.float32).astype(NPBF16)
    wobf = np.asarray(wo, dtype=np.float32).astype(NPBF16)
    bo_bc = np.broadcast_to(np.asarray(bo, np.float32), (P, D)).copy()
    zeros_bc = np.zeros((P, D), np.float32)

    in_maps = []
    for c in range(NCORES):
        b = c // 4
        hs = HPC * HD * (c % 4)
        sl = slice(hs, hs + HPC * HD)
        in_maps.append({
            "xT": np.ascontiguousarray(xbf[b].T),
            "wqT": np.ascontiguousarray(wqbf[sl, :].T),
            "wkT": np.ascontiguousarray(wkbf[sl, :].T),
            "wvT": np.ascontiguousarray(wvbf[sl, :].T),
            "woT": np.ascontiguousarray(wobf[:, sl].T),
            "bq": np.ascontiguousarray(np.asarray(bq, np.float32)[sl].reshape(HPC, P).T),
            "bk": np.ascontiguousarray(np.asarray(bk, np.float32)[sl].reshape(HPC, P).T),
            "bv": np.broadcast_to(np.asarray(bv, np.float32)[sl], (P, HPC * HD)).copy(),
            "bo": bo_bc if c % 4 == 0 else zeros_bc,
            "tri": tri,
            "ident": ident,
        })
    return in_maps


def _classify_mask(mask):
    m = np.asarray(mask, dtype=np.float32).reshape(T, T)
    neg = np.isneginf(m)
    if not neg.any():
        return "full"
    if np.array_equal(neg, np.triu(np.ones((T, T), dtype=bool), k=1)):
        return "causal"
    return "other"


def _numpy_reference(x, mask, wq, bq, wk, bk, wv, bv, wo, bo):
    """Fallback for masks that are not the causal mask."""
    x = np.asarray(x, np.float32)
    m = np.asarray(mask, np.float32).reshape(T, T)
    q = (x.reshape(-1, D) @ np.asarray(wq, np.float32).T + bq).reshape(B, T, H, HD).transpose(0, 2, 1, 3)
    k = (x.reshape(-1, D) @ np.asarray(wk, np.float32).T + bk).reshape(B, T, H, HD).transpose(0, 2, 1, 3)
    v = (x.reshape(-1, D) @ np.asarray(wv, np.float32).T + bv).reshape(B, T, H, HD).transpose(0, 2, 1, 3)
    outh = np.empty((B, H, T, HD), np.float32)
    negm = np.isneginf(m)
    for b in range(B):
        for h in range(H):
            s = (q[b, h] @ k[b, h].T) * SCALE
            s = np.where(negm, -np.inf, s)
            s = s - s.max(axis=-1, keepdims=True)
            e = np.exp(s)
            p = e / e.sum(axis=-1, keepdims=True)
            outh[b, h] = p @ v[b, h]
    o = outh.transpose(0, 2, 1, 3).reshape(B * T, D)
    return (o @ np.asarray(wo, np.float32).T + bo).reshape(B, T, D).astype(np.float32)


def run_spmd(inputs, trace=False, tmpdir=None):
    """Run the device kernel; returns (output [B,T,D] f32, BassKernelResults)."""
    nc = _get_program()
    in_maps = _prep_in_maps(
        inputs["x"], inputs["wq"], inputs["bq"], inputs["wk"], inputs["bk"],
        inputs["wv"], inputs["bv"], inputs["wo"], inputs["bo"])
    kw = {}
    if trace:
        kw = dict(trace=True, tmpdir=tmpdir)
    res = run_bass_kernel_spmd(nc, in_maps, core_ids=list(range(NCORES)), **kw)
    out = np.empty((B, T, D), np.float32)
    for b in range(B):
        acc = np.zeros((T, D), np.float64)
        for c in range(4 * b, 4 * b + 4):
            acc += res.results[c]["out"].astype(np.float64)
        out[b] = acc.astype(np.float32)
    return out, res


def kernel(**inputs) -> np.ndarray:
    if _classify_mask(inputs["mask"]) != "causal":
        return _numpy_reference(**inputs)
    out, _ = run_spmd(inputs)
    return out
